# revision 1
# baseline (speedup 1.0000x reference)
"""Multi-head attention (with the repo's k=q bug) on 8 Trainium2 NeuronCores.

Reference computation (B=2, S=2048, D=512, H=8, DK=64):
    q = query @ Wq.T ; v = value @ Wv.T          (k-projection is dead code)
    qh = q.reshape(B, H, S, DK)  (raw view: head h = a contiguous 256-row slab
                                  of q, re-chunked into rows of 64)
    kh = qh                      (repo bug: key = query.view(...))
    scores = qh @ qh^T / 8 ; mask ; softmax ; x = attn @ vh
    out = x.transpose/reshape @ Wo.T

Sharding: core c owns head h=c for both batches (2 (b,h) pairs/core).

v2 layout: everything bf16 on the matmul path, both batches packed into the
two partition halves (b0 -> partitions 0:64, b1 -> 64:128) so score matmuls
run as concurrent PE quadrant pairs and each exp instruction covers both
batches.  Per (j-tile t, i-chunk n) the device computes, score-transposed:
    E_T = exp(S_T/8 - 20) * tri-mask   (only the diagonal 128-col block of a
                                        diagonal tile needs masking)
    [x_unnorm^T; l] += [vh | ones]^T augmented PV matmul   (per batch)
Chunk epilogue: po = x_unnorm^T.T @ Wo_h.T written bf16; host divides by l
and sums partials over heads/cores.  Fully-masked j-tiles are skipped and
diagonal tiles are column-restricted (causal structure verified on host;
non-causal masks fall back to numpy).
"""

import math
import sys

import numpy as np

sys.path.insert(0, "/opt/trn_rl_repo")

B, S, D, H, DK = 2, 2048, 512, 8, 64
NCORES = 8
SLAB = S // H          # 256 query rows per head-slab
CHUNK = 512            # i-chunk width
JT = 128               # j-tile height
NCHUNK = S // CHUNK    # 4
NJT = S // JT          # 16
KT = D // 128          # 4 k-tiles over the projections' contraction dim
EXP_BIAS = -20.0


def _enable_ldw_opt():
    """Flip walrus --enable-ldw-opt: elides back-to-back LDWEIGHTS of the
    same stationary operand."""
    from concourse import bass_utils
    if getattr(bass_utils, "_ldw_patched", False):
        return
    orig = bass_utils.run_command

    def run_command(argv, **kw):
        import subprocess
        try:
            return orig(argv, **kw)
        except subprocess.CalledProcessError as e:
            err = e.stderr if isinstance(e.stderr, str) else (
                e.stderr.decode() if e.stderr else "")
            out = e.stdout if isinstance(e.stdout, str) else (
                e.stdout.decode() if e.stdout else "")
            sys.stderr.write("WALRUS FAIL STDERR:\n" + err[-4000:] + "\n")
            sys.stderr.write("WALRUS FAIL STDOUT:\n" + out[-4000:] + "\n")
            raise

    bass_utils.run_command = run_command
    bass_utils._ldw_patched = True

_cache: dict = {}


def _build_causal():
    import concourse.bass as bass
    import concourse.tile as tile
    from concourse import bacc, mybir

    _enable_ldw_opt()

    f32 = mybir.dt.float32
    bf16 = mybir.dt.bfloat16
    nc = bacc.Bacc("TRN2", target_bir_lowering=False, debug=False,
                   num_devices=NCORES)

    # inputs (all bf16; batches packed side by side in the free dim)
    qT = nc.dram_tensor("qT", [D, 2 * SLAB], bf16, kind="ExternalInput").ap()
    vT = nc.dram_tensor("vT", [D, 2 * SLAB], bf16, kind="ExternalInput").ap()
    wqT = nc.dram_tensor("wqT", [D, D], bf16, kind="ExternalInput").ap()
    wvT = nc.dram_tensor("wvT", [D, D], bf16, kind="ExternalInput").ap()
    woT = nc.dram_tensor("woT", [DK, D], bf16, kind="ExternalInput").ap()
    mtri = nc.dram_tensor("mtri", [JT, JT], bf16, kind="ExternalInput").ap()
    po = nc.dram_tensor("po", [B, S, D], bf16, kind="ExternalOutput").ap()
    lo = nc.dram_tensor("lo", [B, 1, S], f32, kind="ExternalOutput").ap()

    # qhT holds the head slab j-ordered (col j = 8*r + c); built from the
    # projection via strided repartition copies, phased by row-quarters so
    # early i-chunks start while later rows are still being evacuated.

    with tile.TileContext(nc) as tc:
        with (
            tc.tile_pool(name="const", bufs=1) as constp,
            tc.tile_pool(name="acts", bufs=1) as actp,
            tc.tile_pool(name="qhT", bufs=1) as qhTp,
            tc.tile_pool(name="vh", bufs=1) as vhp,
            tc.tile_pool(name="qc", bufs=1) as qcp,
            tc.tile_pool(name="eT", bufs=4) as eTp,
            tc.tile_pool(name="xT", bufs=2) as xTp,
            tc.tile_pool(name="fo", bufs=2) as fop,
            tc.tile_pool(name="psS", bufs=2, space="PSUM") as psS,
            tc.tile_pool(name="psX", bufs=1, space="PSUM") as psX,
            tc.tile_pool(name="psM", bufs=1, space="PSUM") as psM,
        ):
            # ---- constants / memsets first (cheap engine ops) -----------
            exp_bias = constp.tile([128, 1], f32, tag="ebias")
            nc.gpsimd.memset(exp_bias[:], EXP_BIAS)
            vh_all = []
            for b in range(B):
                t = vhp.tile([128, NJT * (DK + 1)], bf16, tag=f"vha{b}")
                vv = t.rearrange("p (t c) -> p t c", c=DK + 1)
                nc.gpsimd.memset(vv[:, :, DK:DK + 1], 1.0)
                vh_all.append(t)
            vh_v = [t.rearrange("p (t c) -> p t c", c=DK + 1) for t in vh_all]

            # ---- critical-path input DMAs: ONLY wq and qT up front ------
            # (everything downstream waits on a batched DMA semaphore, so
            # keeping wv/vT/wo/mtri out of this batch starts the
            # projections ~5us earlier)
            wq_sb = constp.tile([128, KT * D], bf16, tag="wq")
            qT_sb = actp.tile([128, KT * 2 * SLAB], bf16, tag="qt")
            for h in range(2):
                nc.sync.dma_start(
                    wq_sb.rearrange("p (h c) -> p h c", h=2)[:, h, :]
                    .rearrange("p (k c) -> p k c", k=2),
                    wqT.rearrange("(h k p) c -> h p k c", h=2, k=2)[h])
                nc.scalar.dma_start(
                    qT_sb.rearrange("p (h c) -> p h c", h=2)[:, h, :]
                    .rearrange("p (k c) -> p k c", k=2),
                    qT.rearrange("(h k p) c -> h p k c", h=2, k=2)[h])
            wv_sb = constp.tile([128, KT * D], bf16, tag="wv")
            vT_sb = actp.tile([128, KT * 2 * SLAB], bf16, tag="vt")
            wo_sb = constp.tile([128, D], bf16, tag="wo")
            mt_sb = constp.tile([JT, JT], bf16, tag="mtri")

            # ---- PE warm-up: dummy matmuls into the (still unused) psx
            # banks while the critical input loads are in flight ----------
            dmy = constp.tile([128, D], bf16, tag="dmy")
            nc.gpsimd.memset(dmy[:], 0.0)
            for i in range(16):
                psd = psX.tile([DK + 1, CHUNK], f32, tag=f"psx{i % 2}")
                nc.tensor.matmul(psd[:], dmy[:, 0:DK + 1], dmy[:],
                                 start=True, stop=True)
            wqk = wq_sb.rearrange("p (k c) -> p k c", k=KT)
            wvk = wv_sb.rearrange("p (k c) -> p k c", k=KT)
            qTk = qT_sb.rearrange("p (k c) -> p k c", k=KT)
            vTk = vT_sb.rearrange("p (k c) -> p k c", k=KT)

            # ---- q projection -> qc -> qhT (all-contiguous copies) ------
            qhT = qhTp.tile([128, S], bf16, tag="qhT")
            qhT_v = qhT.rearrange("p (r c) -> p r c", c=H)
            qc_sb = []
            for jg in range(4):
                psq = psM.tile([128, 2 * SLAB], f32, tag=f"psf{jg % 2}")
                for k in range(KT):
                    nc.tensor.matmul(
                        psq[:], wqk[:, k, 128 * jg:128 * (jg + 1)],
                        qTk[:, k, :], start=(k == 0), stop=(k == KT - 1))
                qc = qcp.tile([128, 2 * SLAB], bf16, tag=f"qc{jg}")
                if jg % 2 == 0:
                    nc.vector.tensor_copy(qc[:], psq[:])
                else:
                    nc.scalar.copy(qc[:], psq[:])
                qc_sb.append(qc)

            # ---- remaining input loads, gated behind the critical pair --
            gate_s = constp.tile([1, 1], bf16, tag="gate_s")
            gate_g = constp.tile([1, 1], bf16, tag="gate_g")
            nc.scalar.copy(gate_s[:], qT_sb[0:1, 0:1])
            nc.gpsimd.tensor_copy(gate_g[:], wq_sb[0:1, 0:1])
            for h in range(2):
                nc.gpsimd.dma_start(
                    wv_sb.rearrange("p (h c) -> p h c", h=2)[:, h, :]
                    .rearrange("p (k c) -> p k c", k=2),
                    wvT.rearrange("(h k p) c -> h p k c", h=2, k=2)[h])
                nc.scalar.dma_start(
                    vT_sb.rearrange("p (h c) -> p h c", h=2)[:, h, :]
                    .rearrange("p (k c) -> p k c", k=2),
                    vT.rearrange("(h k p) c -> h p k c", h=2, k=2)[h])
            nc.scalar.dma_start(wo_sb[0:64, :], woT[:, :])
            nc.scalar.dma_start(wo_sb[64:128, :], woT[:, :])
            nc.scalar.dma_start(mt_sb[:], mtri[:, :])

            # ---- v projection (first halves) + vh gathers ---------------
            vsl = {}

            def vproj(rhs):
                for rh in rhs:
                    psv = psM.tile([128, D], f32, tag=f"psf{rh % 2}")
                    b, half = rh // 2, rh % 2
                    for k in range(KT):
                        nc.tensor.matmul(
                            psv[:],
                            vTk[:, k, 256 * b + 128 * half:
                                256 * b + 128 * (half + 1)],
                            wvk[:, k, :], start=(k == 0), stop=(k == KT - 1))
                    vc = actp.tile([128, D], bf16, tag=f"vsl{rh}")
                    nc.vector.tensor_copy(vc[:], psv[:])
                    vsl[rh] = vc

            def vgather(half):
                # dst partition jj = 8*rm + c8 <- vsl[rh][16*tl+rm, 64*c8+d]
                for tl in range(8):
                    for b in range(2):
                        rh = 2 * b + half
                        t_ = 8 * half + tl
                        eng = nc.sync
                        src = vsl[rh].rearrange(
                            "(tl rm) f -> tl rm f", tl=8)[tl]
                        eng.dma_start(vh_v[b][:, t_, 0:DK], src)

            # qc -> qhT strided repartition copies (dst col-stride 8),
            # phased by slab-row range; split across gpsimd and vector
            def qcopy(r0, r1):
                for c in range(8):
                    for b in range(2):
                        dst = qhT_v[64 * b:64 * (b + 1), r0:r1, c]
                        srca = qc_sb[c // 2][
                            64 * (c % 2):64 * (c % 2 + 1),
                            256 * b + r0:256 * b + r1]
                        w = (2 * c + b) % 3
                        if w == 0:
                            nc.gpsimd.tensor_copy(dst, srca)
                        elif w == 1:
                            nc.vector.tensor_copy(dst, srca)
                        else:
                            nc.scalar.copy(dst, srca)
            qcopy(0, 64)
            qcopy(64, 128)
            qcopy(128, 256)

            # lo accumulators (flushed at the very end)
            lacc0 = xTp.tile([1, S], f32, tag="lacc0", bufs=1)
            lacc1 = xTp.tile([1, S], f32, tag="lacc1", bufs=1)

            # ---- attention: epilogues interleaved into next chunk -------
            def make_epilogue(n, psx0, psx1, final=False):
                thunks = []

                def t_evac():
                    xT = xTp.tile([128, CHUNK], bf16, tag="xT")
                    nc.vector.tensor_copy(xT[0:64, :], psx0[0:64, :])
                    if final:
                        nc.scalar.copy(xT[64:128, :], psx1[0:64, :])
                        nc.scalar.copy(
                            lacc1[:, CHUNK * n:CHUNK * (n + 1)],
                            psx1[64:65, :])
                    else:
                        nc.vector.tensor_copy(xT[64:128, :], psx1[0:64, :])
                        nc.vector.tensor_copy(
                            lacc1[:, CHUNK * n:CHUNK * (n + 1)],
                            psx1[64:65, :])
                    nc.vector.tensor_copy(
                        lacc0[:, CHUNK * n:CHUNK * (n + 1)], psx0[64:65, :])
                    if final:
                        nc.sync.dma_start(lo[0, :, :], lacc0[:])
                        nc.gpsimd.dma_start(lo[1, :, :], lacc1[:])
                    fo0 = fop.tile([128, 4 * D], bf16, tag="fo0")
                    fo1 = fop.tile([128, 4 * D], bf16, tag="fo1")
                    make_epilogue.state = (xT, fo0, fo1)
                thunks.append(t_evac)

                def t_proj(u):
                    def run():
                        xT, fo0, fo1 = make_epilogue.state
                        if final and u % 2 == 1:
                            ps = psS.tile([128, 2 * CHUNK], f32, tag="pss")
                            psf0 = ps[:, 0:D]
                            psf1 = ps[:, D:2 * D]
                        else:
                            psf0 = psM.tile([128, D], f32, tag="psf0")
                            psf1 = psM.tile([128, D], f32, tag="psf1")
                        nc.tensor.matmul(
                            psf0[:], xT[0:64, 128 * u:128 * (u + 1)],
                            wo_sb[0:64, :], start=True, stop=True,
                            tile_position=(0, 0))
                        nc.tensor.matmul(
                            psf1[:], xT[64:128, 128 * u:128 * (u + 1)],
                            wo_sb[64:128, :], start=True, stop=True,
                            tile_position=(64, 0))
                        nc.vector.tensor_copy(
                            fo0[:, D * u:D * (u + 1)], psf0[:])
                        if final:
                            nc.scalar.copy(fo1[:, D * u:D * (u + 1)], psf1[:])
                        else:
                            nc.vector.tensor_copy(
                                fo1[:, D * u:D * (u + 1)], psf1[:])
                        if final:
                            r0 = CHUNK * n + 128 * u
                            e0 = nc.sync if u % 2 == 0 else nc.gpsimd
                            e1 = nc.gpsimd if u % 2 == 0 else nc.sync
                            e0.dma_start(
                                po[0, r0:r0 + 128, :], fo0[:, D * u:D * (u + 1)])
                            e1.dma_start(
                                po[1, r0:r0 + 128, :], fo1[:, D * u:D * (u + 1)])
                    return run
                for u in range(4):
                    thunks.append(t_proj(u))

                def t_store():
                    if final:
                        return
                    xT, fo0, fo1 = make_epilogue.state
                    for b, fo in ((0, fo0), (1, fo1)):
                        dst = po[b, CHUNK * n:CHUNK * (n + 1), :].rearrange(
                            "(u p) c -> p u c", u=4)
                        eng = nc.sync if b == 0 else nc.gpsimd
                        eng.dma_start(
                            dst, fo.rearrange("p (u c) -> p u c", u=4))
                thunks.append(t_store)
                return thunks

            pending = []
            for n in (0, 1, 2, 3):
                n_t = 4 * n + 4
                psx0 = psX.tile([DK + 1, CHUNK], f32, tag="psx0")
                psx1 = psX.tile([DK + 1, CHUNK], f32, tag="psx1")
                for t_ in range(n_t):
                    s_ = t_ - 4 * n
                    off = max(0, s_) * JT
                    pss = psS.tile([128, 2 * CHUNK], f32, tag="pss")
                    nc.tensor.matmul(
                        pss[:, off:CHUNK],
                        qhT[0:64, JT * t_:JT * (t_ + 1)],
                        qhT[0:64, CHUNK * n + off:CHUNK * (n + 1)],
                        start=True, stop=True, tile_position=(0, 0))
                    nc.tensor.matmul(
                        pss[:, CHUNK + off:2 * CHUNK],
                        qhT[64:128, JT * t_:JT * (t_ + 1)],
                        qhT[64:128, CHUNK * n + off:CHUNK * (n + 1)],
                        start=True, stop=True, tile_position=(64, 0))
                    eT = eTp.tile([128, 2 * CHUNK], bf16, tag="eT")
                    nc.scalar.activation(
                        eT[:, off:], pss[:, off:],
                        mybir.ActivationFunctionType.Exp,
                        bias=exp_bias[:], scale=1.0 / math.sqrt(DK))
                    if s_ >= 0:
                        sl0 = eT[:, off:off + JT]
                        nc.vector.tensor_mul(sl0, sl0, mt_sb[:])
                        sl1 = eT[:, CHUNK + off:CHUNK + off + JT]
                        nc.gpsimd.tensor_mul(sl1, sl1, mt_sb[:])
                    if n == 0 and t_ == 0:
                        # first-half v projection goes here: after chunk 0's
                        # first score matmul (so exp starts ASAP) but ahead
                        # of the first PV in the PE's program order
                        vproj((0, 2))
                        vgather(0)
                    elif n == 1 and t_ == 0:
                        vproj((3,))
                        vgather(1)
                    nc.tensor.matmul(
                        psx0[:, off:], vh_all[0][:, 65 * t_:65 * t_ + 65],
                        eT[:, off:CHUNK],
                        start=(t_ == 0), stop=(t_ == n_t - 1),
                        skip_group_check=True)
                    nc.tensor.matmul(
                        psx1[:, off:], vh_all[1][:, 65 * t_:65 * t_ + 65],
                        eT[:, CHUNK + off:2 * CHUNK],
                        start=(t_ == 0), stop=(t_ == n_t - 1),
                        skip_group_check=True)
                    if pending:
                        pending.pop(0)()
                if n == 0:
                    # second-half v projection split: rh1 here, rh3 inside
                    # chunk 1's first tile (smaller PE bursts)
                    vproj((1,))
                for th in pending:
                    th()
                pending = make_epilogue(n, psx0, psx1, final=(n == 3))
            for th in pending:
                th()
    nc.compile()
    return nc


def _mask_patterns():
    import ml_dtypes
    p = np.arange(JT)[:, None]
    f = np.arange(JT)[None, :]
    return (p <= f).astype(ml_dtypes.bfloat16)


def _numpy_fallback(query, key, value, mask, Wq, Wk, Wv, Wo):
    q = query @ Wq.T
    v = value @ Wv.T
    qh = q.reshape(B, H, S, DK)
    vh = v.reshape(B, H, S, DK)
    scores = np.einsum("bhqd,bhkd->bhqk", qh, qh) / math.sqrt(DK)
    scores = np.where(mask == 0, np.float32(-1e9), scores)
    scores = scores - scores.max(axis=-1, keepdims=True)
    e = np.exp(scores)
    attn = e / e.sum(axis=-1, keepdims=True)
    x = np.einsum("bhqk,bhkd->bhqd", attn, vh)
    x = x.transpose(0, 2, 1, 3).reshape(B, S, H * DK)
    return (x @ Wo.T).astype(np.float32)


def _run_device(query, value, Wq, Wv, Wo, trace=False):
    import ml_dtypes
    from concourse.bass_utils import run_bass_kernel_spmd

    if "nc" not in _cache:
        _cache["nc"] = _build_causal()
    nc = _cache["nc"]

    bf = ml_dtypes.bfloat16
    mtri = _mask_patterns()
    wqT = np.ascontiguousarray(Wq.T).astype(bf)
    wvT = np.ascontiguousarray(Wv.T).astype(bf)
    in_maps = []
    for c in range(NCORES):
        r0 = SLAB * c
        qs = query[:, r0:r0 + SLAB, :]      # [B, SLAB, D]
        vs = value[:, r0:r0 + SLAB, :]
        in_maps.append({
            # [D, 2*SLAB]: b0 cols then b1 cols
            "qT": np.ascontiguousarray(
                qs.transpose(2, 0, 1).reshape(D, 2 * SLAB)).astype(bf),
            "vT": np.ascontiguousarray(
                vs.transpose(2, 0, 1).reshape(D, 2 * SLAB)).astype(bf),
            "wqT": wqT,
            "wvT": wvT,
            "woT": np.ascontiguousarray(
                Wo[:, DK * c:DK * (c + 1)].T).astype(bf),
            "mtri": mtri,
        })
    res = run_bass_kernel_spmd(nc, in_maps, core_ids=list(range(NCORES)),
                               trace=trace)
    out = np.zeros((B, S, D), dtype=np.float32)
    for c in range(NCORES):
        pc = res.results[c]
        out += pc["po"].astype(np.float32) / \
            pc["lo"].reshape(B, S, 1)
    return out, res


_TRIL = None


def kernel(query, key, value, mask, Wq, Wk, Wv, Wo):
    global _TRIL
    query = np.asarray(query, dtype=np.float32)
    value = np.asarray(value, dtype=np.float32)
    mask = np.asarray(mask)
    Wq = np.asarray(Wq, dtype=np.float32)
    Wv = np.asarray(Wv, dtype=np.float32)
    Wo = np.asarray(Wo, dtype=np.float32)

    if _TRIL is None:
        _TRIL = np.tril(np.ones((S, S), dtype=np.int64))
    m2 = mask.reshape(S, S)
    if not np.array_equal(m2 != 0, _TRIL != 0):
        return _numpy_fallback(query, np.asarray(key), value, mask,
                               Wq, np.asarray(Wk), Wv, Wo)

    out, _ = _run_device(query, value, Wq, Wv, Wo)
    return out

